# revision 7
# baseline (speedup 1.0000x reference)
"""Trainium2 Bass kernel for the leaky CustomRNN:

    xin = x @ W_in                                  [B,T,H]
    h_t = 0.9*h_{t-1} + 0.1*tanh(h_{t-1} @ W_rec + xin_t)
    outputs = hs[1:] @ W_out                        [B,T,O]
    returns (outputs, hidden_states[T+1,B,H])

Strategy (8 NeuronCores, data-parallel over batch B=64 -> 8 per core):
  - Recurrence runs in transposed layout hT[H-partitions, B-free] so the
    per-step elementwise work uses all 128 partitions. Per step:
    z.T = W_rec.T @ h.T via 16 accumulating matmuls (W_rec fp16 stationary)
    plus xin injected into PSUM through an identity-weight matmul.
    tanh on ScalarE (PSUM->SBUF, fp16 out), state update on VectorE with a
    single scalar_tensor_tensor (h_t = 0.1*u + 0.9*h_{t-1}).
  - Full fp16 state history (4 x [128, (T+1)*8]) lives in SBUF; it feeds the
    next step's matmul rhs AND phase 3's stationary operand directly.
  - Phase 1 (x @ W_in) and phase 3 (hs @ W_out) are interleaved into the
    sequential step stream so they hide under the PE-bound recurrence.
  - fp32 copies of the state stream out to DRAM as hsT; the host transposes.
All matmul operands fp16 (fp32 PSUM accumulate): measured end-to-end rel err
vs the fp32 reference ~7e-4.
"""

import numpy as np
from contextlib import ExitStack

import concourse.bass as bass
import concourse.mybir as mybir
import concourse.tile as tile
from concourse.vector_clock import ScopedClock
from concourse.bass_utils import run_bass_kernel_spmd
from concourse.masks import make_identity

ALPHA = 0.1
B, T, I, H, O = 64, 1024, 256, 512, 256
NCORES = 8
BC = B // NCORES          # 8 batch rows per core
HK = H // 128             # 4 h-chunks
IK = I // 128             # 2 i-chunks
NBLK = 64                 # recurrence steps per phase-1 block
NJ = T // NBLK            # 16 phase-1 blocks
P3B = 16                  # steps per phase-3 output block
STG = 32                  # steps per hsT fp32 staging block
F16 = mybir.dt.float16
F32 = mybir.dt.float32
AF = mybir.ActivationFunctionType
ALU = mybir.AluOpType


class PatchedTileContext(tile.TileContext):
    """This toolchain's walrus accepts only ONE sync wait on TPB_CTRL
    instructions (Drain/NOP); Tile's tail drain can carry several. Harvest
    the outstanding waits onto dedicated single-wait SP NOPs instead."""

    def _drain_and_barrier(self, tick_clock, wait_clock):
        sink = self.nc.sync.nop(nofuse=True)
        wait_clock.add_sem_waits(
            sink.ins, ScopedClock({None: tick_clock.global_clock})
        )
        si = sink.ins.sync_info
        waits = list(si.on_wait) if si and si.on_wait else []
        if len(waits) > 1:
            sink.ins.sync_info = mybir.SyncInfo(on_wait=[waits[0]], on_update=[])
            for w in waits[1:]:
                extra = self.nc.sync.nop(nofuse=True)
                extra.ins.sync_info = mybir.SyncInfo(on_wait=[w], on_update=[])
        self.nc.sync.drain()
        self.nc.all_engine_barrier()
        assert self.sems is not None
        popped = self.nc._tile_sem_poison_stack.pop()
        assert popped is self._sem_poison
        self.nc.clear_and_free_semaphores(list(self.sems.allocated().values()))
        self.nc.all_engine_barrier()


def _legalize_single_wait(nc):
    """This toolchain's walrus rejects >1 sync wait on ANY instruction.
    Move surplus waits onto same-engine NOPs inserted immediately before the
    owning instruction (sequencer executes them in order, so wait-A-then-
    wait-B on one engine == wait-(A and B))."""
    n_split = 0
    for bb in nc.main_func.blocks:
        li = bb.instructions
        i = 0
        while i < len(li):
            ins = li[i]
            si = ins.sync_info
            waits = list(si.on_wait) if si and si.on_wait else []
            if len(waits) > 1:
                for j, w in enumerate(waits[:-1]):
                    nop = mybir.InstNoOp(
                        name=nc.get_next_instruction_name(), ins=[], outs=[])
                    nop.engine = ins.engine
                    nop.sync_info = mybir.SyncInfo(on_wait=[w], on_update=[])
                    li.insert(i + j, nop)
                i += len(waits) - 1
                ins.sync_info = mybir.SyncInfo(
                    on_wait=[waits[-1]],
                    on_update=list(si.on_update) if si.on_update else [])
                n_split += 1
            i += 1
    return n_split


def build_bass():
    nc = bass.Bass("TRN2", target_bir_lowering=False, debug=False)
    xT = nc.dram_tensor("xT", [I, T * BC], F16, kind="ExternalInput")
    h0T = nc.dram_tensor("h0T", [H, BC], F16, kind="ExternalInput")
    Wr = nc.dram_tensor("Wr", [H, H], F16, kind="ExternalInput")
    Wi = nc.dram_tensor("Wi", [I, H], F16, kind="ExternalInput")
    Wo = nc.dram_tensor("Wo", [H, O], F16, kind="ExternalInput")
    hsT = nc.dram_tensor("hsT", [H, T * BC], F32, kind="ExternalOutput")
    outO = nc.dram_tensor("outO", [T * BC, O], F32, kind="ExternalOutput")

    with PatchedTileContext(nc) as tc, ExitStack() as ctx:
        consts = ctx.enter_context(tc.tile_pool(name="consts", bufs=1))
        xin_pool = ctx.enter_context(tc.tile_pool(name="xin", bufs=1))
        hist_pool = ctx.enter_context(tc.tile_pool(name="hist", bufs=1))
        xt_pool = ctx.enter_context(tc.tile_pool(name="xt", bufs=2))
        u_pool = ctx.enter_context(tc.tile_pool(name="u", bufs=4))
        hm_pool = ctx.enter_context(tc.tile_pool(name="hm", bufs=4))
        st_pool = ctx.enter_context(tc.tile_pool(name="stg", bufs=2))
        ob_pool = ctx.enter_context(tc.tile_pool(name="ob", bufs=3))
        ps_rec = ctx.enter_context(tc.tile_pool(name="psr", bufs=2, space="PSUM"))
        ps_p1 = ctx.enter_context(tc.tile_pool(name="ps1", bufs=2, space="PSUM"))
        ps_p3 = ctx.enter_context(tc.tile_pool(name="ps3", bufs=2, space="PSUM"))

        # ---- constants / weights
        ident = consts.tile([128, 128], F16, tag="ident", name="ident")
        make_identity(nc, ident[:])
        wr = [consts.tile([128, H], F16, tag=f"wr{k}", name=f"wr{k}") for k in range(HK)]
        for k in range(HK):
            nc.sync.dma_start(out=wr[k][:], in_=Wr[k * 128:(k + 1) * 128, :])
        wi = [consts.tile([128, H], F16, tag=f"wi{k}", name=f"wi{k}") for k in range(IK)]
        for k in range(IK):
            nc.sync.dma_start(out=wi[k][:], in_=Wi[k * 128:(k + 1) * 128, :])
        wo = [consts.tile([128, O], F16, tag=f"wo{k}", name=f"wo{k}") for k in range(HK)]
        for k in range(HK):
            nc.sync.dma_start(out=wo[k][:], in_=Wo[k * 128:(k + 1) * 128, :])

        # ---- fp16 state history: hist[m][:, t*BC:(t+1)*BC] = hT_t chunk m
        hist = [hist_pool.tile([128, (T + 1) * BC], F16, tag=f"hist{m}",
                               name=f"hist{m}")
                for m in range(HK)]
        for m in range(HK):
            nc.sync.dma_start(out=hist[m][:, 0:BC],
                              in_=h0T[m * 128:(m + 1) * 128, :])

        # ---- xin blocks, step-major interleaved: col = s*(HK*BC) + m*BC + b
        xin = [xin_pool.tile([128, NBLK * HK * BC], F16, tag=f"xin{j}", name=f"xin{j}")
               for j in range(NJ)]

        def phase1(j):
            xt = [xt_pool.tile([128, NBLK * BC], F16, tag=f"xt{k}", name=f"xt{k}")
                  for k in range(IK)]
            c0 = j * NBLK * BC
            for k in range(IK):
                nc.sync.dma_start(out=xt[k][:],
                                  in_=xT[k * 128:(k + 1) * 128, c0:c0 + NBLK * BC])
            for m in range(HK):
                ps = ps_p1.tile([128, NBLK * BC], F32, tag="p1", name="p1")
                for k in range(IK):
                    nc.tensor.matmul(ps[:], lhsT=wi[k][:, m * 128:(m + 1) * 128],
                                     rhs=xt[k][:], start=(k == 0), stop=(k == IK - 1))
                dest = xin[j][:].rearrange("p (s g) -> p s g", g=HK * BC)
                dest = dest[:, :, m * BC:(m + 1) * BC]
                src = ps[:].rearrange("p (s b) -> p s b", b=BC)
                if m % 2 == 0:
                    nc.scalar.copy(out=dest, in_=src)
                else:
                    nc.vector.tensor_copy(out=dest, in_=src)

        def phase3(bi):
            ps = ps_p3.tile([128, O], F32, tag="p3", name="p3")
            c0 = bi * P3B * BC + BC    # skip h0 cols
            for k in range(HK):
                nc.tensor.matmul(ps[:], lhsT=hist[k][:, c0:c0 + P3B * BC],
                                 rhs=wo[k][:], start=(k == 0), stop=(k == HK - 1))
            ob = ob_pool.tile([128, O], F32, tag="ob", name="ob")
            nc.scalar.copy(out=ob[:], in_=ps[:])
            r0 = bi * P3B * BC
            nc.sync.dma_start(out=outO[r0:r0 + P3B * BC, :], in_=ob[:])

        def stage_out(w):
            # fp32 copy of hist cols [1 + w*STG .. ] * BC out to DRAM
            c0 = (1 + w * STG) * BC
            d0 = w * STG * BC
            for m in range(HK):
                st = st_pool.tile([128, STG * BC], F32, tag=f"st{m}", name=f"st{m}")
                nc.vector.tensor_copy(out=st[:], in_=hist[m][:, c0:c0 + STG * BC])
                nc.sync.dma_start(
                    out=hsT[m * 128:(m + 1) * 128, d0:d0 + STG * BC], in_=st[:])

        def step(t):
            j, s = (t - 1) // NBLK, (t - 1) % NBLK
            prev = (t - 1) * BC
            cur = t * BC
            hms = [hm_pool.tile([128, BC], F16, tag=f"hm{m}", name=f"hm{m}") for m in range(HK)]
            for m in range(HK):
                nc.vector.tensor_scalar_mul(hms[m][:], hist[m][:, prev:prev + BC],
                                            1.0 - ALPHA)
            for half in range(2):
                ph = ps_rec.tile([128, 2 * BC], F32, tag=f"ph{half}", name=f"ph{half}")
                x0 = s * HK * BC + half * 2 * BC
                nc.tensor.matmul(ph[:], lhsT=ident[:],
                                 rhs=xin[j][:, x0:x0 + 2 * BC],
                                 start=True, stop=False)
                for mi in range(2):
                    m = half * 2 + mi
                    for k in range(HK):
                        nc.tensor.matmul(
                            ph[:, mi * BC:(mi + 1) * BC],
                            lhsT=wr[k][:, m * 128:(m + 1) * 128],
                            rhs=hist[k][:, prev:prev + BC],
                            start=False, stop=(k == HK - 1))
                u = u_pool.tile([128, 2 * BC], F16, tag=f"u{half}", name=f"u{half}")
                nc.scalar.activation(u[:], ph[:], AF.Tanh)
                for mi in range(2):
                    m = half * 2 + mi
                    nc.vector.scalar_tensor_tensor(
                        out=hist[m][:, cur:cur + BC],
                        in0=u[:, mi * BC:(mi + 1) * BC],
                        scalar=ALPHA, in1=hms[m][:],
                        op0=ALU.mult, op1=ALU.add)

        # ---- emission order drives Tile's scheduling priority
        phase1(0)
        for t in range(1, T + 1):
            if (t - 1) % NBLK == 8 and (t - 1) // NBLK + 1 < NJ:
                phase1((t - 1) // NBLK + 1)
            step(t)
            if t % P3B == 0:
                phase3(t // P3B - 1)
            if t % STG == 0:
                stage_out(t // STG - 1)

    _legalize_single_wait(nc)
    return nc


_CACHE = {}


def _get_nc():
    if "nc" not in _CACHE:
        _CACHE["nc"] = build_bass()
    return _CACHE["nc"]


def kernel(x, h, W_in, W_rec, W_out, _trace=False):
    x = np.asarray(x)
    h = np.asarray(h)
    Wi16 = np.asarray(W_in, dtype=np.float16)
    Wr16 = np.asarray(W_rec, dtype=np.float16)
    Wo16 = np.asarray(W_out, dtype=np.float16)

    in_maps = []
    for c in range(NCORES):
        sl = slice(c * BC, (c + 1) * BC)
        xs = x[sl]                                   # [BC, T, I]
        xTc = np.ascontiguousarray(
            xs.transpose(2, 1, 0).reshape(I, T * BC)).astype(np.float16)
        h0Tc = np.ascontiguousarray(h[sl].T).astype(np.float16)
        in_maps.append({"xT": xTc, "h0T": h0Tc,
                        "Wr": Wr16, "Wi": Wi16, "Wo": Wo16})

    nc = _get_nc()
    res = run_bass_kernel_spmd(nc, in_maps, core_ids=list(range(NCORES)),
                               trace=_trace)

    outputs = np.empty((B, T, O), np.float32)
    hidden = np.empty((T + 1, B, H), np.float32)
    hidden[0] = np.asarray(h, dtype=np.float32)
    for c in range(NCORES):
        sl = slice(c * BC, (c + 1) * BC)
        oc = res.results[c]["outO"]                  # [T*BC, O], row=(t-1)*BC+b
        outputs[sl] = oc.reshape(T, BC, O).transpose(1, 0, 2)
        hc = res.results[c]["hsT"]                   # [H, T*BC], col=(t-1)*BC+b
        hidden[1:, sl, :] = hc.reshape(H, T, BC).transpose(1, 2, 0)
    kernel._last_exec_time_ns = res.exec_time_ns
    kernel._last_res = res
    return outputs, hidden
